# revision 35
# baseline (speedup 1.0000x reference)
"""Trainium2 Bass kernel for nn_Attention_54262616817926.

kernel(x, w_qkv, b_qkv, w_proj, b_proj) -> out [8, 4, 1024, 192] float32.

Sharding: pure data-parallel over batch B=8 across the 8 NeuronCores
(core c computes batch c end-to-end; no collectives). Inputs are
preprocessed host-side (transposed/augmented layouts); see _body for the
on-chip dataflow.

Self-contained: inlines the TileContext tail-drain workaround and the
kernel builder; hardcodes B=8, P=4, N=1024, d=192, H=4.
"""

"""Workaround for walrus 'Too many sync wait commands' on the TileContext
tail drain: this build's walrus accepts at most 1 sync wait on a TPB_CTRL
(Drain) instruction, but TileContext._drain_and_barrier packs every
outstanding semaphore wait onto one drain. Split them into one wait-carrying
nop per semaphore, then emit a clean drain."""

import bass_rust
import concourse.mybir as mybir
import concourse.tile as tile
from concourse.vector_clock import ScopedClock

_WAIT_OP = {
    "ge": "sem-ge",
    "sem-ge": "sem-ge",
}


def _patched_drain_and_barrier(self, tick_clock, wait_clock):
    nc = self.nc
    dummy = mybir.InstNoOp(
        name=f"I-tailwaits-{nc.next_id()}",
        engine=mybir.EngineType.SP,
        ins=[],
        outs=[],
    )
    wait_clock.add_sem_waits(dummy, ScopedClock({None: tick_clock.global_clock}))
    waits = list(dummy.sync_info.on_wait) if dummy.sync_info is not None else []
    for w in waits:
        sem = bass_rust.SemaphoreHandle(w.ant_name, w.id)
        op = _WAIT_OP.get(str(w.wait_mode), "sem-ge")
        nc.sync.nop().wait_op(sem, w.wait_value, op)

    nc.sync.drain()

    nc.all_engine_barrier()
    assert self.sems is not None
    popped = nc._tile_sem_poison_stack.pop()
    assert popped is self._sem_poison
    nc.clear_and_free_semaphores(list(self.sems.allocated().values()))
    nc.all_engine_barrier()


def split_multi_waits(nc):
    """Post-pass over finished BIR: walrus accepts at most one sync wait per
    TPB_CTRL instruction, but Tile's loop reset/exit blocks pack several.
    Replace each multi-wait instruction's waits with per-wait NoOps inserted
    immediately before it on the same engine."""
    for f in nc.m.functions:
        for bb in f.blocks:
            insts = bb.instructions
            if not any(i.sync_info is not None and len(i.sync_info.on_wait) > 1
                       for i in insts):
                continue
            out = []
            for inst in insts:
                si = inst.sync_info
                if si is not None and len(si.on_wait) > 1:
                    waits = list(si.on_wait)
                    for w in waits:
                        out.append(mybir.InstNoOp(
                            name=f"I-splitw-{nc.next_id()}",
                            engine=inst.engine,
                            ins=[],
                            outs=[],
                            sync_info=mybir.SyncInfo(on_wait=[w],
                                                     on_update=[]),
                            bass_nofuse=True,
                        ))
                    inst.sync_info = mybir.SyncInfo(
                        on_wait=[], on_update=list(si.on_update))
                out.append(inst)
            bb.instructions = out


tile.TileContext._drain_and_barrier = _patched_drain_and_barrier




from contextlib import ExitStack

import numpy as np

import concourse.bass as bass
import concourse.tile as tile
from concourse import library_config, mybir

FP = mybir.dt.float32
BF = mybir.dt.bfloat16
FR = mybir.dt.float32r

EMBED_DIM = 192
NUM_HEADS = 4
HEAD_DIM = EMBED_DIM // NUM_HEADS  # 48
SCALE = HEAD_DIM ** -0.5


# ---------------------------------------------------------------- host prep

def prep_weights(w_qkv, b_qkv, w_proj, b_proj):
    """Host-side weight preprocessing (shared by all cores).

    Returns dict of numpy arrays:
      wqk [193, 512]: per f-chunk of 128: [h0(48) pad(16) h1(48) pad(16)],
                      chunks = [q01, q23, k01, k23]; row 192 = bias row.
                      Q part (incl bias) pre-scaled by 1/sqrt(D).
      wv  [193, 192]: [Wv^T; b_v]
      wp  [192, 192]: Wp^T
      bp  [1, 192]  : b_proj
    """
    d = EMBED_DIM
    wq = w_qkv[0:d] * SCALE          # [192,192] rows = q features
    bq = b_qkv[0:d] * SCALE
    wk = w_qkv[d:2 * d]
    bk = b_qkv[d:2 * d]
    wv = w_qkv[2 * d:3 * d]
    bv = b_qkv[2 * d:3 * d]

    def chunk2(w, b, h0, h1):
        # [193, 128] column block: head h0 cols 0-47, zeros 48-63,
        # head h1 cols 64-111, zeros 112-127; last row = bias.
        blk = np.zeros((d + 1, 128), dtype=np.float32)
        blk[0:d, 0:48] = w[h0 * 48:(h0 + 1) * 48].T
        blk[d, 0:48] = b[h0 * 48:(h0 + 1) * 48]
        blk[0:d, 64:112] = w[h1 * 48:(h1 + 1) * 48].T
        blk[d, 64:112] = b[h1 * 48:(h1 + 1) * 48]
        return blk

    wqk = np.concatenate(
        [chunk2(wq, bq, 0, 1), chunk2(wq, bq, 2, 3),
         chunk2(wk, bk, 0, 1), chunk2(wk, bk, 2, 3)], axis=1)  # [193, 512]

    wv_aug = np.concatenate([wv.T, bv[None, :]], axis=0)  # [193, 192]
    # proj weights in z^T-padded order: head h occupies rows h*64..h*64+47,
    # rows h*64+48..h*64+63 are zero (match zT pad rows).
    wp_pad = np.zeros((256, 192), dtype=np.float32)
    for h in range(4):
        wp_pad[h * 64:h * 64 + 32] = w_proj.T[h * 48:h * 48 + 32]
        wp_pad[h * 64 + 33:h * 64 + 49] = w_proj.T[h * 48 + 32:(h + 1) * 48]
    bp = np.ascontiguousarray(b_proj[None, :])            # [1, 192]
    return {
        "wqk": np.ascontiguousarray(wqk, dtype=np.float32),
        "wv": np.ascontiguousarray(wv_aug, dtype=np.float32),
        "wp": wp_pad,
        "bp": bp.astype(np.float32),
    }


def prep_x_core(x_core):
    """x_core [P, N, d] -> xT [d+1, P*N] with ones row appended."""
    P, N, d = x_core.shape
    xt = np.ascontiguousarray(x_core.reshape(P * N, d).T, dtype=np.float32)
    return np.concatenate([xt, np.ones((1, P * N), dtype=np.float32)], axis=0)


# ---------------------------------------------------------------- kernel

def build_nc(P_loc=4, N=1024, repeat=1, phases=("qkv", "attn", "proj"),
             attn_parts=("s", "exp", "pv", "norm"), dtype_mode="fp32",
             debug_taps=False, nonce=77):
    d, H, D = EMBED_DIM, NUM_HEADS, HEAD_DIM
    T = P_loc * N
    NK = N // 128            # key tiles per (p, h)
    QC = -(-N // 512)        # q chunks per pair (ceil)
    qcs = [(i * 512, min(512, N - i * 512)) for i in range(QC)]
    TT = T // 128            # token tiles
    TCH = -(-T // 512)       # token 512-chunks
    tcs = [(i * 512, min(512, T - i * 512)) for i in range(TCH)]

    DT = {"bf16": BF, "fp32r": FR}.get(dtype_mode, FP)
    nc = bass.Bass()
    xT = nc.dram_tensor("xT", [d + 1, T], DT, kind="ExternalInput")
    wqk = nc.dram_tensor("wqk", [d + 1, 512], DT, kind="ExternalInput")
    wv = nc.dram_tensor("wv", [d + 1, 192], DT, kind="ExternalInput")
    wp = nc.dram_tensor("wp", [256, 192], DT, kind="ExternalInput")
    bp = nc.dram_tensor("bp", [1, 192], FP, kind="ExternalInput")
    # The remote executable cache keys on the I/O signature only (not BIR
    # content); this size-varying dummy input forces a distinct cache slot
    # per kernel revision.
    nc.dram_tensor("nonce", [1, nonce], FP, kind="ExternalInput")
    out = nc.dram_tensor("out", [T, 192], FP, kind="ExternalOutput")
    dbg = None
    if debug_taps:
        dbg = {
            "qkT": nc.dram_tensor("dbg_qkT", [128, T], DT,
                                  kind="ExternalOutput"),
            "v": nc.dram_tensor("dbg_v", [128, 256], DT,
                                kind="ExternalOutput"),
            "pt": nc.dram_tensor("dbg_pt", [128, N], DT,
                                 kind="ExternalOutput"),
            "z": nc.dram_tensor("dbg_z", [128, N], FP,
                                kind="ExternalOutput"),
            "r": nc.dram_tensor("dbg_r", [16, 128], FP,
                                kind="ExternalOutput"),
            "l16": nc.dram_tensor("dbg_l16", [16, 128], FP,
                                  kind="ExternalOutput"),
            "rb": nc.dram_tensor("dbg_rb", [16, 128], FP,
                                 kind="ExternalOutput"),
            "zT": nc.dram_tensor("dbg_zT", [128, N], DT,
                                 kind="ExternalOutput"),
        }

    with tile.TileContext(nc) as tc:
        if repeat > 1:
            with tc.For_i(0, repeat, 1):
                _body(nc, tc, xT, wqk, wv, wp, bp, out,
                      P_loc, N, T, NK, qcs, TT, tcs, phases, attn_parts,
                      dtype_mode)
        else:
            _body(nc, tc, xT, wqk, wv, wp, bp, out,
                  P_loc, N, T, NK, qcs, TT, tcs, phases, attn_parts,
                  dtype_mode, dbg)
    return nc


def _split_multi_waits(nc):
    """Post-pass: walrus accepts at most one sync wait per TPB_CTRL
    instruction, but Tile's loop reset/exit blocks pack several. Replace each
    multi-wait instruction's waits with per-wait NoOps inserted before it."""
    for f in nc.m.functions:
        for bb in f.blocks:
            insts = bb.instructions
            if not any(i.sync_info is not None and len(i.sync_info.on_wait) > 1
                       for i in insts):
                continue
            out = []
            for inst in insts:
                si = inst.sync_info
                if si is not None and len(si.on_wait) > 1:
                    for w in list(si.on_wait):
                        out.append(mybir.InstNoOp(
                            name=f"I-splitw-{nc.next_id()}",
                            engine=inst.engine,
                            ins=[],
                            outs=[],
                            sync_info=mybir.SyncInfo(on_wait=[w],
                                                     on_update=[]),
                            bass_nofuse=True,
                        ))
                    inst.sync_info = mybir.SyncInfo(
                        on_wait=[], on_update=list(si.on_update))
                out.append(inst)
            bb.instructions = out


def _body(nc, tc, xT, wqk, wv, wp, bp, out, P_loc, N, T, NK, qcs, TT, tcs,
          phases=("qkv", "attn", "proj"), attn_parts=("s", "exp", "pv", "norm"),
          dtype_mode="fp32", dbg=None):
    d = EMBED_DIM
    DT = {"bf16": BF, "fp32r": FR}.get(dtype_mode, FP)
    mk = lambda ap: ap
    with ExitStack() as ctx:
        persist = ctx.enter_context(tc.tile_pool(name="persist", bufs=1))

        # ---- phase 0: load inputs
        wqk_hi = persist.tile([128, 512], DT, tag="wqk_hi")
        wqk_lo = persist.tile([65, 512], DT, tag="wqk_lo")
        nc.sync.dma_start(out=wqk_hi, in_=wqk[0:128, :])
        nc.sync.dma_start(out=wqk_lo, in_=wqk[128:193, :])

        wv_hi = persist.tile([128, 192], DT, tag="wv_hi")
        wv_lo = persist.tile([65, 192], DT, tag="wv_lo")
        nc.sync.dma_start(out=wv_hi, in_=wv[0:128, :])
        nc.sync.dma_start(out=wv_lo, in_=wv[128:193, :])

        wp_hi = persist.tile([128, 192], DT, tag="wp_hi")
        wp_lo = persist.tile([128, 192], DT, tag="wp_lo")
        nc.sync.dma_start(out=wp_hi, in_=wp[0:128, :])
        nc.sync.dma_start(out=wp_lo, in_=wp[128:256, :])

        bp_sb = persist.tile([128, 192], FP, tag="bp_sb")
        nc.sync.dma_start(out=bp_sb, in_=bp[:].to_broadcast([128, 192]))

        # qkv^T buffers: QA(h0,h1) QB(h2,h3) KA(h0,h1) KB(h2,h3); head pair
        # layout: first head rows 0-47, second head rows 64-111.
        qkT = [persist.tile([128, T], DT, tag=f"qkT{i}", name=f"qkT{i}")
               for i in range(4)]

        # V token-major, per (token-tile, head): 48 cols + ones col + pad.
        # V strips per (token-tile, head): cols 0-31 = V dims 0-31, col 32 =
        # ones (-> l row at aligned partition 32), cols 33-48 = V dims 32-47,
        # cols 49-63 zero. wp_pad rows match this permutation.
        v_sb = persist.tile([128, TT, 4, 64], DT, tag="v_sb")
        # memset on float32r trips an ISA check; write through a same-width
        # bitcast view (fp32r shares the fp32 bit layout)
        MDT = FP if DT == FR else DT
        nc.vector.memset(v_sb[:].bitcast(MDT), 0.0)
        nc.vector.memset(v_sb[:, :, :, 32:33].bitcast(MDT), 1.0)

        # z^T accumulators in padded layout: tile hg holds heads (2hg, 2hg+1)
        # at partition rows 0-47 / 64-111; pad rows zeroed once (proj weights
        # have matching zero rows, but garbage could be inf/nan).
        zT0 = persist.tile([128, T], DT, tag="zT0")
        zT1 = persist.tile([128, T], DT, tag="zT1")

        # ---- phases 1+2. bf16 fits x^T in SBUF alongside everything, so no
        # scoped pool (a scoped pool's released zone would make the attention
        # pools' tiles wait for the whole qkv phase).
        with ExitStack() as qctx:
            xT_pool = persist
            qkv_ps = qctx.enter_context(
                tc.tile_pool(name="qkv_ps", bufs=2, space="PSUM"))
            xT_hi = xT_pool.tile([128, T], DT, tag="xT_hi")
            xT_lo = xT_pool.tile([65, T], DT, tag="xT_lo")
            # pair-0 slice first so its qkv matmuls start early
            nc.sync.dma_start(out=xT_hi[:, 0:1024], in_=xT[0:128, 0:1024])
            nc.sync.dma_start(out=xT_lo[:, 0:1024], in_=xT[128:193, 0:1024])
            nc.sync.dma_start(out=xT_hi[:, 1024:T], in_=xT[0:128, 1024:T])
            nc.sync.dma_start(out=xT_lo[:, 1024:T], in_=xT[128:193, 1024:T])
            # token-chunk-major so pair p=0's Q/K/V slices land first and
            # attention can start while later chunks still compute; copies
            # alternate DVE/ACT to halve the drain serialization.
            for ic, (c0, cw) in enumerate(tcs if "qkv" in phases else []):
                # q01/k01 chunks gate pair-0's first S matmuls; emit them
                # ahead of q23/k23 within each token chunk
                for fc in (0, 2, 1, 3):
                    ps = qkv_ps.tile([128, 512], FP, tag="qk", name="ps_qk")
                    nc.tensor.matmul(ps[:, 0:cw],
                                     lhsT=mk(wqk_hi[:, fc * 128:(fc + 1) * 128]),
                                     rhs=mk(xT_hi[:, c0:c0 + cw]),
                                     start=True, stop=False)
                    nc.tensor.matmul(ps[:, 0:cw],
                                     lhsT=mk(wqk_lo[:, fc * 128:(fc + 1) * 128]),
                                     rhs=mk(xT_lo[:, c0:c0 + cw]),
                                     start=False, stop=True)
                    if ic < 2 and fc >= 2:
                        # ACT is idle pre-attention; parallel copy chains
                        # halve the latency to pair-0's first S matmul
                        nc.scalar.copy(qkT[fc][:, c0:c0 + cw], ps[:, 0:cw])
                    else:
                        nc.vector.tensor_copy(qkT[fc][:, c0:c0 + cw],
                                              ps[:, 0:cw])

        # ---- phase 3: attention per (pair p, head-group hg)
        with ExitStack() as actx:
            s_pool = actx.enter_context(
                tc.tile_pool(name="s_ps", bufs=1, space="PSUM"))
            z_pool = actx.enter_context(
                tc.tile_pool(name="z_ps", bufs=2, space="PSUM"))
            pt_pool = actx.enter_context(tc.tile_pool(name="pt", bufs=4))
            sm_pool = actx.enter_context(tc.tile_pool(name="sm", bufs=3))
            dr_pool = actx.enter_context(
                tc.tile_pool(name="dr", bufs=3, space="DRAM"))
            ob_pool = actx.enter_context(tc.tile_pool(name="ob", bufs=4))

            def emit_v_tiles(lo, hi):
                if "qkv" not in phases:
                    return
                for tt in range(lo, hi):
                    ps = z_pool.tile([128, 192], FP, tag="z", name="ps_v")
                    sl = slice(tt * 128, (tt + 1) * 128)
                    nc.tensor.matmul(ps, lhsT=mk(xT_hi[:, sl]),
                                     rhs=mk(wv_hi[:]),
                                     start=True, stop=False)
                    nc.tensor.matmul(ps, lhsT=mk(xT_lo[:, sl]),
                                     rhs=mk(wv_lo[:]),
                                     start=False, stop=True)
                    psh = ps.rearrange("p (h dd) -> p h dd", h=4)
                    nc.vector.tensor_copy(v_sb[:, tt, :, 0:32],
                                          psh[:, :, 0:32])
                    if tt < 8:
                        nc.scalar.copy(v_sb[:, tt, :, 33:49],
                                       psh[:, :, 32:48])
                    else:
                        nc.vector.tensor_copy(v_sb[:, tt, :, 33:49],
                                              psh[:, :, 32:48])

            NPT = N // 128
            emit_v_tiles(0, NPT)
            n_pairs = P_loc if "attn" in phases else 0
            for p in range(n_pairs):
                if p > 0:
                    emit_v_tiles(p * NPT, (p + 1) * NPT)
                poff = p * N
                for hg in range(2):
                    QA, KA = qkT[hg], qkT[2 + hg]
                    zps = z_pool.tile([128, N], FP, tag="z")
                    def emit_pv(pv):
                        if pv is None:
                            return
                        kt_, pA, pB = pv
                        ci_ = p * NK + kt_
                        for (q0, qw) in qcs:
                            nc.tensor.matmul(
                                zps[0:64, q0:q0 + qw],
                                lhsT=mk(v_sb[:, ci_, 2 * hg, 0:64]),
                                rhs=mk(pA[:, q0:q0 + qw]),
                                start=(kt_ == 0), stop=(kt_ == NK - 1),
                                skip_group_check=True)
                            nc.tensor.matmul(
                                zps[64:128, q0:q0 + qw],
                                lhsT=mk(v_sb[:, ci_, 2 * hg + 1, 0:64]),
                                rhs=mk(pB[:, q0:q0 + qw]),
                                start=(kt_ == 0), stop=(kt_ == NK - 1),
                                skip_group_check=True)

                    pend = None
                    for kt in range(NK):
                        koff = poff + kt * 128
                        sA = s_pool.tile([128, N], FP, tag="sA")
                        sB = s_pool.tile([128, N], FP, tag="sB")
                        if "s" in attn_parts:
                            for (q0, qw) in qcs:
                                nc.tensor.matmul(
                                    sA[:, q0:q0 + qw],
                                    lhsT=mk(KA[0:48, koff:koff + 128]),
                                    rhs=mk(QA[0:48, poff + q0:poff + q0 + qw]),
                                    start=True, stop=True)
                                nc.tensor.matmul(
                                    sB[:, q0:q0 + qw],
                                    lhsT=mk(KA[64:112, koff:koff + 128]),
                                    rhs=mk(QA[64:112, poff + q0:poff + q0 + qw]),
                                    start=True, stop=True)
                        ptA = pt_pool.tile([128, N], DT, tag="ptA")
                        ptB = pt_pool.tile([128, N], DT, tag="ptB")
                        if "exp" in attn_parts:
                            nc.scalar.activation(
                                ptA, sA, mybir.ActivationFunctionType.Exp)
                            nc.scalar.activation(
                                ptB, sB, mybir.ActivationFunctionType.Exp)
                        # PV deferred one kt so exp(kt+1)'s S matmuls reach
                        # the PE queue before PV(kt): ACT and PE overlap
                        # instead of ping-ponging.
                        if "pv" in attn_parts:
                            emit_pv(pend)
                            pend = (kt, ptA, ptB)
                    if "pv" in attn_parts:
                        emit_pv(pend)

                    if "norm" not in attn_parts:
                        continue
                    # drain: recip(l rows) -> broadcast -> scale into zT
                    # l rows sit at partitions 32 (head A) / 96 (head B).
                    # Standard ops only: copy rows out of PSUM, DMA-reshape to
                    # [16,128] so the iterative reciprocal's free dim is short,
                    # then 0-stride DMA broadcasts.
                    NC8 = N // 128
                    lrowA = sm_pool.tile([1, N], FP, tag="lr", name="lrowA")
                    lrowB = sm_pool.tile([1, N], FP, tag="lr", name="lrowB")
                    nc.vector.tensor_copy(lrowA, zps[32:33, :])
                    nc.vector.tensor_copy(lrowB, zps[96:97, :])
                    ldr = dr_pool.tile([2, N], FP, tag="ldr", name="ldr")
                    nc.sync.dma_start(out=ldr[0:1, :], in_=lrowA[:])
                    nc.sync.dma_start(out=ldr[1:2, :], in_=lrowB[:])
                    l16 = sm_pool.tile([2 * NC8, 128], FP, tag="l16",
                                       name="l16")
                    nc.sync.dma_start(
                        out=l16[:],
                        in_=ldr[:].rearrange("t (c q) -> (t c) q", q=128))
                    r16 = sm_pool.tile([2 * NC8, 128], FP, tag="r16",
                                       name="r16")
                    nc.vector.reciprocal(out=r16, in_=l16)
                    rdr = dr_pool.tile([2, N], FP, tag="rdr", name="rdr")
                    nc.sync.dma_start(
                        out=rdr[:].rearrange("t (c q) -> (t c) q", q=128),
                        in_=r16[:])
                    bcA = sm_pool.tile([64, N], FP, tag="bc", name="bcA")
                    bcB = sm_pool.tile([64, N], FP, tag="bc", name="bcB")
                    nc.sync.dma_start(out=bcA[:],
                                      in_=rdr[0:1, :].to_broadcast([64, N]))
                    nc.sync.dma_start(out=bcB[:],
                                      in_=rdr[1:2, :].to_broadcast([64, N]))
                    zsl = slice(poff, poff + N)
                    zTt = zT0 if hg == 0 else zT1
                    nc.vector.tensor_mul(zTt[0:64, zsl], zps[0:64, :], bcA[:])
                    nc.vector.tensor_mul(zTt[64:128, zsl],
                                         zps[64:128, :], bcB[:])
                    if dbg and p == 0 and hg == 0:
                        zcopy = sm_pool.tile([128, N], FP, tag="zc",
                                             name="zcopy")
                        nc.vector.tensor_copy(zcopy, zps[:])
                        nc.sync.dma_start(out=dbg["z"][:], in_=zcopy[:])
                        nc.sync.dma_start(out=dbg["r"][:], in_=r16[:])
                        nc.sync.dma_start(out=dbg["l16"][:], in_=l16[:])
                        rb = sm_pool.tile([2 * NC8, 128], FP, tag="rb",
                                          name="rb")
                        nc.vector.reciprocal(out=rb, in_=l16)
                        nc.sync.dma_start(out=dbg["rb"][:], in_=rb[:])
                        nc.sync.dma_start(out=dbg["zT"][:], in_=zT0[:, 0:N])

        # ---- phase 4: proj + bias (after attention pools release; its
        # psum waits only on long-done events, and sharing z slots during
        # attention serializes everything behind drain-paced releases)
        with tc.tile_pool(name="pj_ps", bufs=4, space="PSUM") as pj_pool, \
                tc.tile_pool(name="ob2", bufs=4) as ob2_pool:
            for tt in range(TT if "proj" in phases else 0):
                sl = slice(tt * 128, (tt + 1) * 128)
                ps = pj_pool.tile([128, 192], FP, tag="pj", name="ps_pj")
                nc.tensor.matmul(ps, lhsT=mk(zT0[:, sl]), rhs=mk(wp_hi[:]),
                                 start=True, stop=False)
                nc.tensor.matmul(ps, lhsT=mk(zT1[:, sl]), rhs=mk(wp_lo[:]),
                                 start=False, stop=True)
                ob = ob2_pool.tile([128, 192], FP, tag="ob", name="ob")
                nc.vector.tensor_add(ob, ps, bp_sb)
                nc.sync.dma_start(out=out[sl, :], in_=ob)


# ---------------------------------------------------------------- runner

def make_in_maps(x, w_qkv, b_qkv, w_proj, b_proj, n_cores=8,
                 dtype_mode="fp32", nonce=77):
    import ml_dtypes
    w = prep_weights(np.asarray(w_qkv), np.asarray(b_qkv),
                     np.asarray(w_proj), np.asarray(b_proj))
    if dtype_mode == "bf16":
        for k in ("wqk", "wv", "wp"):
            w[k] = w[k].astype(ml_dtypes.bfloat16)
    x = np.asarray(x)
    in_maps = []
    for c in range(n_cores):
        m = dict(w)
        xt = prep_x_core(x[c])
        if dtype_mode == "bf16":
            xt = xt.astype(ml_dtypes.bfloat16)
        m["xT"] = xt
        m["nonce"] = np.zeros((1, nonce), dtype=np.float32)
        in_maps.append(m)
    return in_maps


def assemble_out(results, B, P, N, d):
    outs = [results[c]["out"].reshape(P, N, d) for c in range(B)]
    return np.stack(outs, axis=0)



# ------------------------------------------------------------------ entry

DTYPE_MODE = "bf16"
NONCE = 171
B = 8

_CACHED = {}


def _get_nc(repeat=1):
    key = (DTYPE_MODE, repeat)
    if key not in _CACHED:
        nc = build_nc(P_loc=4, N=1024, repeat=repeat,
                      dtype_mode=DTYPE_MODE,
                      nonce=NONCE + (1 if repeat > 1 else 0))
        _split_multi_waits(nc)
        _CACHED[key] = nc
    return _CACHED[key]


def build_nc_cached(repeat=1):
    return _get_nc(repeat)


def _make_in_maps(inputs):
    return make_in_maps(inputs["x"], inputs["w_qkv"], inputs["b_qkv"],
                        inputs["w_proj"], inputs["b_proj"], n_cores=B,
                        dtype_mode=DTYPE_MODE, nonce=NONCE)


def build_nc_repeat(repeat):
    return build_nc(P_loc=4, N=1024, repeat=repeat, dtype_mode=DTYPE_MODE)


def kernel(x, w_qkv, b_qkv, w_proj, b_proj):
    from concourse.bass_utils import run_bass_kernel_spmd

    in_maps = make_in_maps(x, w_qkv, b_qkv, w_proj, b_proj, n_cores=B,
                           dtype_mode=DTYPE_MODE, nonce=NONCE)
    nc = _get_nc()
    res = run_bass_kernel_spmd(nc, in_maps, core_ids=list(range(B)))
    outs = [res.results[c]["out"].reshape(4, 1024, EMBED_DIM)
            for c in range(B)]
    return np.stack(outs, axis=0).astype(np.float32)

